# revision 50
# baseline (speedup 1.0000x reference)
"""Trainium2 Bass kernel for nn_ModalityConsisLoss (8 NeuronCores).

Reference computation:
    v_spa/v_seq = concat([f[:,a,:], f[:,2,:]], -1) @ W + b   for a in (0,1,3)  -> [3B, D]
    z = normalize_rows(concat([v_spa, v_seq]))               -> [6B, D]
    sim = z @ z.T ;  pos = diag pairs (i, i+3B)
    loss = sum(-pos/T) + sum(log(rowsum(exp(sim/T)) - diag)) / (6B)

Strategy: data-parallel over B with a rank-rotated symmetric-block sim
schedule.  Each core owns 768 spa rows + 768 seq rows (part r).  The sim
matrix decomposes into A = spa x spa, B = seq x seq (both symmetric) and
C = spa x seq.  Core r computes, in core-relative slot order (slot k =
part (r+k) mod 8, materialized via rank-dependent DMA offsets from the
AllGather output):
    A slots 0..4, B slots 0..4, C slots 0..7
Row sums come from the ACT accumulator; the missing transposed blocks'
row sums are recovered as COLUMN sums of the computed blocks (slots 1..3
of A/B and all C slots), scattered into a [8,2,768] buffer at rotated
offsets and summed by one ReduceScatter whose rank-r shard is exactly
core r's rows.  Slot 4 of A/B (the antipodal pair) is computed by both
members of the pair, so it needs no column sums and is scheduled after
the ReduceScatter issues to hide its latency.  Slot-0 (diagonal) blocks
need no collective at all and run during the AllGather windows.
Column sums use fp8 e-values with DoubleRow ones-matmuls contracting an
ib-pair (256 rows) per instruction.
"""
import sys
from contextlib import ExitStack

sys.path.insert(0, "/opt/trn_rl_repo")

import numpy as np

import concourse.bass as bass
import concourse.mybir as mybir
import concourse.tile as tile
from concourse import bacc
from concourse import bass_utils
from concourse.ap import AP
from concourse.masks import make_identity

F32 = mybir.dt.float32
BF16 = mybir.dt.bfloat16
FP8 = mybir.dt.float8e4
AF = mybir.ActivationFunctionType
ALU = mybir.AluOpType
DR = mybir.MatmulPerfMode.DoubleRow

N_CORES = 8
B = 2048
BL = B // N_CORES          # 256 local batch rows
D = 512
KB = D // 128              # 4 d blocks of 128
HROWS = 3 * BL             # 768 rows per modality half
LROWS = 2 * HROWS          # 1536 local z-rows (spa 768 | seq 768)
R = N_CORES * LROWS        # 12288 total rows
HIB = 6                    # row blocks of 128 per modality half
LH = (0, 1, 3)             # left heads of the pairs (x, 2)
TEMP = 0.5
ZSCALE = 16.0              # fp8 z scaling
ESCALE = (1.0 / TEMP) / (ZSCALE * ZSCALE)
POS_COEF = (-2.0 / TEMP) / (ZSCALE * ZSCALE)
E2 = float(np.exp(2.0))    # diagonal term exp(2 * ||z||^2), ||z|| == 1
INV_COUNT = 1.0 / R
NSTAT = 9                  # stats cols/ib: A0 A12 A3 A4 C0 C12 C34 C56 C7
AGB = 512 * HROWS          # elements per rank block in ag_out


def _body(ctx, nc, tc, f_aps, w_ap, b_ap, out_ap):
    const_pool = ctx.enter_context(tc.tile_pool(name="const", bufs=1))
    small_pool = ctx.enter_context(tc.tile_pool(name="small", bufs=1))
    vt_pool = ctx.enter_context(tc.tile_pool(name="vt", bufs=1))
    dram_pool = ctx.enter_context(tc.tile_pool(name="dram", bufs=1,
                                               space="DRAM"))
    big_pool = ctx.enter_context(tc.tile_pool(name="big", bufs=1))

    # warm-up operands first: the PE ramp (HAM) gates the whole prologue
    warm_sb = const_pool.tile([128, 256], BF16)
    nc.vector.memset(warm_sb[:], 0.0)
    ident = const_pool.tile([128, 128], F32)
    make_identity(nc, ident[:])
    ones_col = const_pool.tile([128, 1], F32)
    nc.vector.memset(ones_col[:], 1.0)
    ones_row = const_pool.tile([1, 128], F32)
    nc.vector.memset(ones_row[:], 1.0)
    ones_dr = const_pool.tile([128, 2, 16], FP8)
    nc.vector.memset(ones_dr[:], 1.0)
    neg_e2 = const_pool.tile([128, 1], F32)
    nc.vector.memset(neg_e2[:], -E2)
    # preload the ln/exp table set during the idle startup window so the
    # norm chain (which gates the AllGather issue) doesn't pay the load
    warm_act = const_pool.tile([1, 1], F32)
    nc.vector.memset(warm_act[:], 1.0)
    nc.scalar.activation(warm_act[:], warm_act[:], AF.Ln)
    nc.scalar.activation(warm_act[:], warm_act[:], AF.Exp)

    # b columns: [128, 4] (per d_out block)
    b_col = const_pool.tile([128, 4], F32)
    for m in range(KB):
        nc.sync.dma_start(b_col[:, m:m + 1], b_ap[m * 128:(m + 1) * 128])
    w_bf = const_pool.tile([128, 8, D], BF16)

    vT = vt_pool.tile([128, KB, LROWS], F32)       # [d_out(blk,128), rows]
    zT_loc = small_pool.tile([128, KB, LROWS], FP8, tag="zT_loc")
    r_row = small_pool.tile([1, LROWS], F32, tag="r_row")
    stats = small_pool.tile([128, 2 * HIB * NSTAT], F32, tag="stats")
    nc.vector.memset(stats[:], 0.0)
    colacc_spa = small_pool.tile([1, 3 * HROWS], F32, tag="cacc_spa")
    nc.vector.memset(colacc_spa[:], 0.0)
    colacc_seq = small_pool.tile([1, 8 * HROWS], F32, tag="cacc_seq")
    nc.vector.memset(colacc_seq[:], 0.0)
    zero_sb = small_pool.tile([128, 48], F32, tag="zero_sb")
    nc.vector.memset(zero_sb[:], 0.0)
    colden = small_pool.tile([128, 2 * HIB], F32, tag="colden")

    # shared DRAM for the collectives
    ag_in = [None, None]
    ag_out = [None, None]
    for mod in range(2):
        ag_in[mod] = dram_pool.tile([4 * 128, HROWS], FP8,
                                    name=f"ag_in{mod}", tag=f"ag_in{mod}")
        ag_out[mod] = dram_pool.tile([N_CORES * 4 * 128, HROWS], FP8,
                                     addr_space="Shared",
                                     name=f"ag_out{mod}", tag=f"ag_out{mod}")
    rs_in = [dram_pool.tile([N_CORES * HROWS], F32, tag=f"rs_in{m}",
                            name=f"rs_in{m}") for m in range(2)]
    rs_out = [dram_pool.tile([HROWS], F32, tag=f"rs_out{m}",
                             name=f"rs_out{m}") for m in range(2)]

    # gathered z, core-relative slots: spa slots 1..4, seq slots 1..7
    zT_spa = big_pool.tile([128, KB, 4 * HROWS], FP8, tag="zT_spa")
    zT_seq = big_pool.tile([128, KB, 7 * HROWS], FP8, tag="zT_seq")

    def scol(ib, j):
        return ib * NSTAT + j

    def sim_mms(ps, ib, rhs_src, rhs_base, width):
        """DR matmuls for one chunk: rows ib*128.., cols rhs_base..+width."""
        widths = (512,) * (width // 512) + ((width % 512,) if width % 512 else ())
        for g in range(2):
            lhsT = zT_loc[:, 2 * g:2 * g + 2, ib * 128:(ib + 1) * 128]
            lo = 0
            for w in widths:
                nc.tensor.matmul(
                    ps[:, lo:lo + w], lhsT=lhsT,
                    rhs=rhs_src[:, 2 * g:2 * g + 2, rhs_base + lo:rhs_base + lo + w],
                    start=(g == 0), stop=(g == 1), perf_mode=DR)
                lo += w

    def exp_chunk(ps, width, ib, j, e_out=None):
        out = e_out if e_out is not None else ps[:, 0:width]
        sc = scol(ib, j)
        nc.scalar.activation(out, ps[:, 0:width], AF.Exp, scale=ESCALE,
                             accum_out=stats[:, sc:sc + 1])

    def colsum_dr(e_ap, pc):
        """Column sums of an ib-pair: e_ap [128, 2, 768] fp8 -> pc [1, 768]."""
        for lo, w in ((0, 512), (512, 256)):
            nc.tensor.matmul(pc[:, lo:lo + w], lhsT=ones_dr[:, :, 0:1],
                             rhs=e_ap[:, :, lo:lo + w],
                             start=True, stop=True, perf_mode=DR)

    def cacc_add(cacc, seg, pc):
        sl = slice(seg * HROWS, (seg + 1) * HROWS)
        nc.vector.tensor_add(cacc[:, sl], cacc[:, sl], pc[:, 0:HROWS])

    ec0_pool = ctx.enter_context(tc.tile_pool(name="ec0", bufs=3))
    eC0_tiles = []

    with tc.tile_pool(name="fstage", bufs=4) as fst_pool, \
         tc.tile_pool(name="ftrans", bufs=1) as ft_pool, \
         tc.tile_pool(name="sq", bufs=2) as sq_pool, \
         tc.tile_pool(name="ps_t", bufs=2, space="PSUM") as ps_t, \
         tc.tile_pool(name="ps_px", bufs=2, space="PSUM") as ps_px, \
         tc.tile_pool(name="ps_sq", bufs=1, space="PSUM") as ps_sq, \
         tc.tile_pool(name="ps_rb", bufs=1, space="PSUM") as ps_rb:

        # PE warm-up: HAM holds the PE at 1.2 GHz until ~3.4us of SUSTAINED
        # activity (a full 4096-cycle window of real matmuls). 40 N=128
        # matmuls give ~4.3us of back-to-back PE work; they overlap the f
        # DMA wait. A scrap copy + WAW DMA to out keeps the chain live.
        wps = ps_t.tile([128, 128], F32, name="wps", tag="pst")
        # one accumulation group: back-to-back MMs with no inter-MM drain,
        # so the HAM activity window sees ~100% PE duty and un-throttles
        for i in range(56):
            nc.tensor.matmul(wps[:], lhsT=warm_sb[:, 0:128],
                             rhs=warm_sb[:, 0:128], start=(i == 0),
                             stop=(i == 55))
        scrap = const_pool.tile([1, 1], F32)
        nc.vector.tensor_copy(scrap[:], wps[0:1, 0:1])
        nc.sync.dma_start(out_ap[:], scrap[:])

        def keepalive():
            # tiny real matmul to keep the HAM activity window non-idle
            # while the PE runs transposes (which don't count as activity)
            kps = ps_t.tile([128, 128], F32, name="wps", tag="pst")
            nc.tensor.matmul(kps[:, 0:64], lhsT=warm_sb[:, 0:128],
                             rhs=warm_sb[:, 0:64], start=True, stop=True)

        # f loads, one DMA per (mod, half, head) so the first transposes can
        # start as soon as the first head lands; mod-0 loads go first, W in
        # between (needed by proj-0), mod-1 after.
        f_sts = {}

        def load_f(mod):
            for a in (0, 2, 1, 3):         # heads 0+2 first: proj-pa0 needs them
                for h in range(2):
                    f_st = fst_pool.tile([128, D], F32,
                                         name=f"f_st{mod}{h}{a}", tag="f_st",
                                         bufs=8)
                    nc.sync.dma_start(
                        f_st[:], f_aps[mod][h * 128:(h + 1) * 128, a, :])
                    f_sts[(mod, h, a)] = f_st

        load_f(0)
        # W: [1024, 512] f32 -> bf16 [128, 8(kblk), 512(d_out)]
        w_st = fst_pool.tile([128, 8, D], F32, tag="w_st", bufs=1)
        for kb in range(8):
            nc.sync.dma_start(w_st[:, kb, :], w_ap[kb * 128:(kb + 1) * 128, :])
        load_f(1)
        nc.vector.tensor_copy(w_bf[:], w_st[:])

        # rs_in zero fill (no deps; must complete before the slot writes)
        for m in range(2):
            nc.sync.dma_start(rs_in[m].rearrange("(p x) -> p x", p=128),
                              zero_sb[:])

        def drain(gen):
            for _ in gen:
                pass

        def step(gen, n):
            for _ in range(n):
                next(gen, None)

        fTs = {}

        def tr_steps(mod, act_copies, heads=(0, 2, 1, 3)):
            """Generator: one f transpose + PSUM->SBUF copy per next()."""
            if mod not in fTs:
                fTs[mod] = ft_pool.tile([128, 4, KB, 2 * 128], BF16,
                                        name=f"fT{mod}", tag=f"fT{mod}")
            fT = fTs[mod]
            tcount = 0
            for a in heads:
                for h in range(2):
                    f_st = f_sts[(mod, h, a)]
                    for kb in range(KB):
                        pst = ps_t.tile([128, 128], F32, name="pst",
                                        tag="pst")
                        nc.tensor.transpose(
                            pst[:], f_st[:, kb * 128:(kb + 1) * 128],
                            ident[:])
                        dst = fT[:, a, kb, h * 128:(h + 1) * 128]
                        if act_copies and tcount % 2 == 0:
                            nc.scalar.copy(dst, pst[:])
                        else:
                            nc.vector.tensor_copy(dst, pst[:])
                        tcount += 1
                        if tcount % 4 == 0:
                            keepalive()
                        yield

        # v_pa = f[head LH[pa]] @ W_top + (f[head 2] @ W_bot + b): the second
        # term is shared by all three pairs -- compute it once per modality
        # (bias folded in), then each pair is 4 matmuls + one DVE add.
        commons = {}

        def proj_common(mod):
            fT = fTs[mod]
            com = small_pool.tile([128, KB, 256], F32, name=f"com{mod}",
                                  tag=f"com{mod}")
            commons[mod] = com
            for m in range(KB):
                psv = ps_px.tile([128, HROWS], F32, name="psv", tag="px")
                for kb in range(KB):
                    nc.tensor.matmul(
                        psv[:, 0:256],
                        lhsT=w_bf[:, 4 + kb, m * 128:(m + 1) * 128],
                        rhs=fT[:, 2, kb, :],
                        start=(kb == 0), stop=(kb == KB - 1))
                nc.scalar.add(com[:, m, :], psv[:, 0:256], b_col[:, m:m + 1])

        def proj_pa(mod, pa):
            c0 = mod * HROWS
            fT = fTs[mod]
            for m in range(KB):
                psv = ps_px.tile([128, HROWS], F32, name="psv", tag="px")
                for kb in range(KB):
                    nc.tensor.matmul(
                        psv[:, 0:256],
                        lhsT=w_bf[:, kb, m * 128:(m + 1) * 128],
                        rhs=fT[:, LH[pa], kb, :],
                        start=(kb == 0), stop=(kb == KB - 1))
                col0 = c0 + pa * 256
                nc.vector.tensor_add(vT[:, m, col0:col0 + 256],
                                     psv[:, 0:256], commons[mod][:, m, :])

        def tp_interleaved(mod, act_copies):
            """T(head 2) -> shared W_bot term (warms the HAM with real
            matmuls) -> T(head 0) -> pa0 -> T(1) -> pa1 -> T(3) -> pa2."""
            drain(tr_steps(mod, act_copies, heads=(2,)))
            proj_common(mod)
            drain(tr_steps(mod, act_copies, heads=(0,)))
            proj_pa(mod, 0)
            drain(tr_steps(mod, act_copies, heads=(1,)))
            proj_pa(mod, 1)
            drain(tr_steps(mod, act_copies, heads=(3,)))
            proj_pa(mod, 2)

        def norm_publish(mod, filler=None):
            """ssq -> r -> zT_loc -> AllGather. The PE ops here are gated by
            DVE/ACT latencies; `filler` (a transpose generator) is stepped
            between them to keep the PE queue dense."""
            c0 = mod * HROWS

            def fill(n):
                if filler is not None:
                    step(filler, n)

            # ---- norms: ssq over d for this half's 768 columns ----
            ssq = small_pool.tile([1, HROWS], F32, name=f"ssq{mod}",
                                  tag=f"ssq{mod}")
            for co, cw in ((0, 512), (512, 256)):
                ps_ssq = ps_sq.tile([1, 512], F32, name="ps_ssq", tag="sq")
                for m in range(KB):
                    sq = sq_pool.tile([128, 512], F32, name="sq", tag="sq")
                    nc.vector.tensor_mul(sq[:, :cw],
                                         vT[:, m, c0 + co:c0 + co + cw],
                                         vT[:, m, c0 + co:c0 + co + cw])
                    nc.tensor.matmul(ps_ssq[:, :cw], lhsT=ones_col[:],
                                     rhs=sq[:, :cw],
                                     start=(m == 0), stop=(m == KB - 1))
                    fill(2)
                nc.vector.tensor_copy(ssq[:, co:co + cw], ps_ssq[:, :cw])

            # r = ZSCALE / sqrt(ssq) = exp(-0.5 * ln(ssq / ZSCALE^2)):
            # both Ln and Exp live in the natural_log_exp table set (as does
            # the sim Exp and the final Ln), so no ACT table-set thrash.
            lssq = small_pool.tile([1, HROWS], F32, name=f"lssq{mod}",
                                   tag=f"lssq{mod}")
            nc.scalar.activation(lssq[:], ssq[:], AF.Ln, 0.0,
                                 1.0 / (ZSCALE * ZSCALE))
            nc.scalar.activation(r_row[:, c0:c0 + HROWS], lssq[:], AF.Exp,
                                 0.0, -0.5)
            fill(4)

            # zT_loc half = fp8(vT * r)
            for co, cw in ((0, 512), (512, 256)):
                rb = ps_rb.tile([128, 512], F32, name="rb", tag="rb")
                nc.tensor.matmul(rb[:, :cw], lhsT=ones_row[:],
                                 rhs=r_row[:, c0 + co:c0 + co + cw],
                                 start=True, stop=True)
                for m in range(KB):
                    nc.vector.tensor_mul(
                        zT_loc[:, m, c0 + co:c0 + co + cw],
                        vT[:, m, c0 + co:c0 + co + cw], rb[:, :cw])
                    fill(2)

            # ---- publish this half: AllGather ----
            nc.sync.dma_start(
                ag_in[mod].rearrange("(m p) c -> p m c", p=128),
                zT_loc[:, :, c0:c0 + HROWS])
            nc.gpsimd.collective_compute(
                "AllGather", ALU.bypass,
                replica_groups=[list(range(N_CORES))],
                ins=[ag_in[mod].opt()], outs=[ag_out[mod].opt()])

        def diag_chunk(ib, mod, j, e_out=None):
            # slot-0 sim block: rows ib vs the local block of modality `mod`
            ps = ps_px.tile([128, HROWS], F32, name="psd", tag="px")
            sim_mms(ps, ib, zT_loc, mod * HROWS, HROWS)
            exp_chunk(ps, HROWS, ib, j, e_out=e_out)

        # ============ prologue, stage-interleaved across mods ============
        # PE order keeps dense real-MM work available at every point: the
        # mod-1 transposes fill the (DVE-gated) mod-0 norm chain so the spa
        # AllGather publishes as early as possible; phase A then runs inside
        # the AllGather(seq) window. Diagonal blocks fill the norm chains.
        tp_interleaved(0, act_copies=True)
        g1 = tr_steps(1, act_copies=False)
        norm_publish(0, filler=g1)         # + AllGather(spa)
        drain(g1)
        proj_common(1)
        for pa in range(3):
            proj_pa(1, pa)
        for ib in range(3):                # A0: spa x spa diag, rowsum only
            diag_chunk(ib, 0, 0)

        def a0_filler():
            for ib in range(3, HIB):
                diag_chunk(ib, 0, 0)
                yield

        norm_publish(1, filler=a0_filler())   # + AllGather(seq)
        # C0: spa x seq diag (colsums done in the main scope); B0: seq diag
        for ibp in range(3):
            eC0 = ec0_pool.tile([128, 2, HROWS], FP8, name=f"eC0_{ibp}",
                                tag="eC0")
            eC0_tiles.append(eC0)
            for ip in range(2):
                ib = 2 * ibp + ip
                diag_chunk(ib, 1, 4, e_out=eC0[:, ip, :])
        for ib in range(HIB, 2 * HIB):     # B0: seq x seq diag
            diag_chunk(ib, 1, 0)

        # ---- pos_i = r_i * r_{i+768} * sum_d vT[d, i] * vT[d, i+768] ----
        pos_raw = small_pool.tile([1, HROWS], F32, tag="pos_raw")
        for co, cw in ((0, 512), (512, 256)):
            ps_pp = ps_sq.tile([1, 512], F32, name="ps_pp", tag="sq")
            for m in range(KB):
                pp = sq_pool.tile([128, 512], F32, name="pp", tag="sq")
                nc.vector.tensor_mul(pp[:, :cw], vT[:, m, co:co + cw],
                                     vT[:, m, HROWS + co:HROWS + co + cw])
                nc.tensor.matmul(ps_pp[:, :cw], lhsT=ones_col[:],
                                 rhs=pp[:, :cw],
                                 start=(m == 0), stop=(m == KB - 1))
            nc.vector.tensor_copy(pos_raw[:, co:co + cw], ps_pp[:, :cw])
        rrp = small_pool.tile([1, HROWS], F32, tag="rrp")
        nc.vector.tensor_mul(rrp[:], r_row[:, 0:HROWS], r_row[:, HROWS:LROWS])
        pos_row = small_pool.tile([1, HROWS], F32, tag="pos_row")
        nc.vector.tensor_mul(pos_row[:], pos_raw[:], rrp[:])
        pos_sum = small_pool.tile([1, 1], F32, tag="pos_sum")
        nc.vector.tensor_reduce(pos_sum[:], pos_row[:],
                                axis=mybir.AxisListType.X, op=ALU.add)

        # ---- rotated gather: slot k <- rank (pid+k) % 8 ----
        pid = nc.sync.partition_id()
        for mod, dst, nslot in ((0, zT_spa, 4), (1, zT_seq, 7)):
            base = ag_out[mod][0:512, :].rearrange("(m p) c -> p m c", p=128)
            for k in range(1, nslot + 1):
                idx = (pid + k) % N_CORES
                src = AP(base.tensor, idx * AGB, base.ap)
                nc.sync.dma_start(
                    dst[:, :, (k - 1) * HROWS:k * HROWS], src)

    # ================= main sim phases =================
    with tc.tile_pool(name="ps_sim", bufs=2, space="PSUM") as ps_sim, \
         tc.tile_pool(name="ps_pc", bufs=1, space="PSUM") as ps_pc, \
         tc.tile_pool(name="eA", bufs=2) as eA_pool, \
         tc.tile_pool(name="eB", bufs=2) as eB_pool, \
         tc.tile_pool(name="eC", bufs=2) as eC_pool:

        # C0 (diagonal) column sums from the prologue-scope e tiles
        for eC0 in eC0_tiles:
            pc = ps_pc.tile([1, HROWS], F32, name="pc", tag="pc")
            colsum_dr(eC0[:, :, :], pc)
            cacc_add(colacc_seq, 0, pc)

        def phase(name, ib0, rhs_src, e_pool, e_slots, pairs, single_slot,
                  jbase, cacc, seg_of):
            """One off-diagonal phase (A, B, or C slots with colsums)."""
            for ibp in range(3):
                e = e_pool.tile([128, 2, e_slots * HROWS], FP8,
                                name=f"e{name}{ibp}", tag=f"e{name}")
                for ip in range(2):
                    ib = ib0 + 2 * ibp + ip
                    j = jbase
                    eoff = 0
                    for (sa, sb) in pairs:
                        ps = ps_sim.tile([128, 1536], F32, name="ps_sim",
                                         tag="ps_sim")
                        sim_mms(ps, ib, rhs_src, (sa - 1) * HROWS, 1536)
                        exp_chunk(ps, 1536, ib, j,
                                  e_out=e[:, ip, eoff:eoff + 1536])
                        eoff += 1536
                        j += 1
                    if single_slot is not None:
                        ps = ps_sim.tile([128, 1536], F32, name="ps_sim",
                                         tag="ps_sim")
                        sim_mms(ps, ib, rhs_src,
                                (single_slot - 1) * HROWS, HROWS)
                        exp_chunk(ps, HROWS, ib, j,
                                  e_out=e[:, ip, eoff:eoff + HROWS])
                for si in range(e_slots):
                    pc = ps_pc.tile([1, HROWS], F32, name="pc", tag="pc")
                    colsum_dr(e[:, :, si * HROWS:(si + 1) * HROWS], pc)
                    cacc_add(cacc, seg_of(si), pc)

        pid2 = nc.sync.partition_id()

        def scatter_rs(mod, cacc, segs):
            # write each colacc segment to its owner rank's shard, then
            # ReduceScatter: rank r's output shard = its rows' missing sums
            rbase = rs_in[mod][0:HROWS]
            for k in segs:
                idx = (pid2 + k) % N_CORES
                dst = AP(rbase.tensor, idx * HROWS, rbase.ap)
                sseg = (k - segs[0]) if mod == 0 else k
                nc.sync.dma_start(
                    dst, cacc[:, sseg * HROWS:(sseg + 1) * HROWS])
            nc.gpsimd.collective_compute(
                "ReduceScatter", ALU.add,
                replica_groups=[list(range(N_CORES))],
                ins=[rs_in[mod].opt()], outs=[rs_out[mod].opt()])

        # A: spa rows x spa slots (1,2) paired + 3 single, colsums 1..3
        # (phase A runs inside the AllGather(seq) window)
        phase("A", 0, zT_spa, eA_pool, 3, [(1, 2)], 3, 1,
              colacc_spa, lambda si: si)
        # spa ReduceScatter issues now and hides under phases B and C
        scatter_rs(0, colacc_spa, [1, 2, 3])
        # B: seq rows x seq slots (1,2) + 3, colsums into seq segs 1..3
        phase("B", HIB, zT_seq, eB_pool, 3, [(1, 2)], 3, 1,
              colacc_seq, lambda si: si + 1)
        # C: spa rows x seq slots (1,2),(3,4),(5,6) + 7, colsums segs 1..7
        phase("C", 0, zT_seq, eC_pool, 7, [(1, 2), (3, 4), (5, 6)], 7, 5,
              colacc_seq, lambda si: si + 1)
        scatter_rs(1, colacc_seq, list(range(8)))

        # ---- slot-4 (antipodal) blocks overlap the seq ReduceScatter ----
        for ib in range(HIB):
            ps = ps_sim.tile([128, 1536], F32, name="ps_sim", tag="ps_sim")
            sim_mms(ps, ib, zT_spa, 3 * HROWS, HROWS)
            exp_chunk(ps, HROWS, ib, 3)
        for ib in range(HIB, 2 * HIB):
            ps = ps_sim.tile([128, 1536], F32, name="ps_sim", tag="ps_sim")
            sim_mms(ps, ib, zT_seq, 3 * HROWS, HROWS)
            exp_chunk(ps, HROWS, ib, 3)

        # colden: rank-r shard of each ReduceScatter
        nc.sync.dma_start(colden[:, 0:HIB],
                          rs_out[0][:].rearrange("(i p) -> p i", p=128))
        nc.sync.dma_start(colden[:, HIB:2 * HIB],
                          rs_out[1][:].rearrange("(i p) -> p i", p=128))

    # ---------- final reduction ----------
    with tc.tile_pool(name="ps_fin", bufs=1, space="PSUM") as ps_fin:
        denom = small_pool.tile([128, 2 * HIB], F32, tag="denom")
        nc.vector.tensor_reduce(
            denom[:], stats.rearrange("p (i x) -> p i x", x=NSTAT),
            axis=mybir.AxisListType.X, op=ALU.add)
        nc.vector.tensor_add(denom[:], denom[:], colden[:])
        logd = small_pool.tile([128, 2 * HIB], F32, tag="logd")
        nc.scalar.activation(logd[:], denom[:], AF.Ln, bias=neg_e2[:])
        logsum = small_pool.tile([128, 1], F32, tag="logsum")
        nc.vector.tensor_reduce(logsum[:], logd[:],
                                axis=mybir.AxisListType.X, op=ALU.add)
        fin = ps_fin.tile([1, 1], F32, tag="fin")
        nc.tensor.matmul(fin[:], lhsT=ones_col[:], rhs=logsum[:],
                         start=True, stop=True)
        res = small_pool.tile([1, 1], F32, tag="res")
        # res = (pos_sum * POS_COEF + sum(log denom)) / R
        nc.vector.scalar_tensor_tensor(res[:], pos_sum[:], POS_COEF,
                                       fin[:], op0=ALU.mult, op1=ALU.add)
        nc.vector.tensor_scalar_mul(res[:], res[:], INV_COUNT)
        nc.sync.dma_start(out_ap[:], res[:])


_NC_CACHE = None


def build_nc():
    global _NC_CACHE
    if _NC_CACHE is not None:
        return _NC_CACHE
    nc = bacc.Bacc("TRN2", target_bir_lowering=False, debug=False,
                   num_devices=N_CORES)
    f_spa = nc.dram_tensor("f_spa", [BL, 4, D], F32, kind="ExternalInput").ap()
    f_seq = nc.dram_tensor("f_seq", [BL, 4, D], F32, kind="ExternalInput").ap()
    w_ap = nc.dram_tensor("W", [2 * D, D], F32, kind="ExternalInput").ap()
    b_ap = nc.dram_tensor("b", [D], F32, kind="ExternalInput").ap()
    out_ap = nc.dram_tensor("out", [1, 1], F32, kind="ExternalOutput").ap()
    with tile.TileContext(nc) as tc, ExitStack() as ctx:
        _body(ctx, nc, tc, (f_spa, f_seq), w_ap, b_ap, out_ap)
    nc.compile()
    _NC_CACHE = nc
    return nc


def run(inputs, **kw):
    nc = build_nc()
    f_seq = np.ascontiguousarray(np.asarray(inputs["f_seq"], dtype=np.float32))
    f_spa = np.ascontiguousarray(np.asarray(inputs["f_spa"], dtype=np.float32))
    W = np.ascontiguousarray(np.asarray(inputs["W"], dtype=np.float32))
    b = np.ascontiguousarray(np.asarray(inputs["b"], dtype=np.float32))
    in_maps = []
    for c in range(N_CORES):
        sl = slice(c * BL, (c + 1) * BL)
        in_maps.append({"f_seq": np.ascontiguousarray(f_seq[sl]),
                        "f_spa": np.ascontiguousarray(f_spa[sl]),
                        "W": W, "b": b})
    res = None
    last_err = None
    for attempt in range(4):
        try:
            res = bass_utils.run_bass_kernel_spmd(
                nc, in_maps, core_ids=list(range(N_CORES)), **kw)
            break
        except Exception as e:
            # the axon terminal occasionally reports a transient
            # "device unrecoverable" on first attach; retry after a pause
            last_err = e
            import time
            time.sleep(20 * (attempt + 1))
    if res is None:
        raise last_err
    total = np.float64(0.0)
    for c in range(N_CORES):
        total += np.float64(res.results[c]["out"][0, 0])
    return np.float32(total), res


def kernel(**inputs) -> np.ndarray:
    loss, _ = run(inputs)
    return np.asarray(loss, dtype=np.float32)


if __name__ == "__main__":
    rng = np.random.default_rng(0)
    inputs = {
        "f_seq": rng.standard_normal((B, 4, D), dtype=np.float32),
        "f_spa": rng.standard_normal((B, 4, D), dtype=np.float32),
        "W": (rng.standard_normal((2 * D, D), dtype=np.float32) * 0.02),
        "b": np.zeros((D,), dtype=np.float32),
    }
    print(kernel(**inputs))
